# revision 4
# baseline (speedup 1.0000x reference)
"""Bidirectional 2-layer GRU encoder on 8 Trainium2 NeuronCores.

Sharding: data-parallel over batch (B=64 -> 8 per core); each core runs both
directions of both layers for its batch slice. No collectives.

Per-core dataflow (all "T" tensors are feature-major = transposed):
  phase A(l): batched input projection x_gates^T = W_ih @ x^T, gate-major
              output, staged to DRAM in 64-step pieces.
  phase B(l): recurrence. Per step per direction:
              gh = h^T @ W_hh (fp16 col-tiled strip matmuls, PSUM fp32)
              -> ACT copy PSUM->SBUF fp16
              -> 3 full-128 PE transposes -> gh^T (gate-major, PSUM fp16)
              -> fused elementwise (DVE/ACT) -> h' (fp32 master + fp16 copy)
  Host pre-transposes x and weights; host un-transposes outputs.
"""
import numpy as np

B_TOT, S, I_IN, H, L, D = 64, 512, 256, 512, 2, 2
NCORES = 8
BL = B_TOT // NCORES          # 8 batch rows per core
G3 = 3 * H                    # 1536
NCH = H // 128                # 4 h-chunks
NJ = G3 // 128                # 12 gate blocks (j = g*4 + c)
PIECE = 64                    # recurrence steps per staged piece
NP = S // PIECE               # 8 pieces
PB = PIECE * BL               # 512 cols per piece

_CACHE = {}


def _build():
    import concourse.bacc as bacc
    import concourse.mybir as mybir
    from concourse.tile import TileContext

    f16 = mybir.dt.float16
    f32 = mybir.dt.float32

    nc = bacc.Bacc("TRN2", target_bir_lowering=False, debug=False)

    # ---------------- tensors ----------------
    xT_in = nc.dram_tensor("xT", [I_IN, S * BL], f16, kind="ExternalInput")
    whh_in = {}
    wih_in = {}
    bx_in = {}
    for l in range(L):
        kin = I_IN if l == 0 else D * H
        for d in range(D):
            whh_in[l, d] = nc.dram_tensor(f"whh_{l}_{d}", [H, G3], f16, kind="ExternalInput")
            wih_in[l, d] = nc.dram_tensor(f"wih_{l}_{d}", [kin, G3], f16, kind="ExternalInput")
            bx_in[l, d] = nc.dram_tensor(f"bx_{l}_{d}", [128, NJ], f32, kind="ExternalInput")
    ident_in = nc.dram_tensor("ident", [128, 128], f16, kind="ExternalInput")

    y1T_out = nc.dram_tensor("y1T", [D, H, S * BL], f32, kind="ExternalOutput")
    hN_out = nc.dram_tensor("hN", [L * D, H, BL], f32, kind="ExternalOutput")

    # internal DRAM
    xg = {}
    for l in range(L):
        for d in range(D):
            xg[l, d] = nc.dram_tensor(f"xg_{l}_{d}", [NP, G3, PB], f16)
    y0T = {d: nc.dram_tensor(f"y0T_{d}", [H, S * BL], f16) for d in range(D)}

    Sig = mybir.ActivationFunctionType.Sigmoid
    Tanh = mybir.ActivationFunctionType.Tanh
    Ident = mybir.ActivationFunctionType.Identity

    with TileContext(nc) as tc:
        with (
            tc.tile_pool(name="const", bufs=1) as cpool,
            tc.tile_pool(name="hstate", bufs=2) as hpool,
            tc.tile_pool(name="scratch", bufs=3) as spool,
            tc.tile_pool(name="ystage", bufs=2) as ypool,
            tc.tile_pool(name="xwin", bufs=2) as xwpool,
        ):
            ident = cpool.tile([128, 128], f16, tag="ident")
            nc.sync.dma_start(out=ident, in_=ident_in[:, :])
            bx = {}
            for l in range(L):
                for d in range(D):
                    t = cpool.tile([128, NJ], f32, tag=f"bx{l}{d}")
                    nc.sync.dma_start(out=t, in_=bx_in[l, d][:, :])
                    bx[l, d] = t

            for l in range(L):
                KIN = 2 if l == 0 else 8
                # -------- phase A: input projection for layer l --------
                with (
                    tc.tile_pool(name=f"wih{l}", bufs=1) as wpool,
                    tc.tile_pool(name=f"rp{l}", bufs=2) as rpool,
                    tc.tile_pool(name=f"xst{l}", bufs=4) as xstpool,
                    tc.tile_pool(name=f"psA{l}", bufs=4, space="PSUM") as psA,
                ):
                    wih = {}
                    for d in range(D):
                        w = wpool.tile([128, KIN * G3], f16, tag=f"wih{d}")
                        nc.sync.dma_start(
                            out=w.rearrange("p (k n) -> p k n", k=KIN),
                            in_=wih_in[l, d].rearrange("(k p) n -> p k n", p=128),
                        )
                        wih[d] = w
                    for p in range(NP):
                        rp = rpool.tile([128, KIN * 512], f16, tag="rp")
                        rp3 = rp.rearrange("p (k n) -> p k n", k=KIN)
                        if l == 0:
                            nc.sync.dma_start(
                                out=rp3,
                                in_=xT_in.rearrange("(k p) n -> p k n", p=128)[
                                    :, :, p * PB : (p + 1) * PB
                                ],
                            )
                        else:
                            for d2 in range(D):
                                nc.sync.dma_start(
                                    out=rp3[:, d2 * 4 : (d2 + 1) * 4, :],
                                    in_=y0T[d2].rearrange("(k p) n -> p k n", p=128)[
                                        :, :, p * PB : (p + 1) * PB
                                    ],
                                )
                        for d in range(D):
                            w3 = wih[d].rearrange("p (k n) -> p k n", k=KIN)
                            for m in range(NJ):
                                ps = psA.tile([128, 512], f32, tag="psA")
                                for k in range(KIN):
                                    nc.tensor.matmul(
                                        ps,
                                        w3[:, k, m * 128 : (m + 1) * 128],
                                        rp3[:, k, :],
                                        start=(k == 0),
                                        stop=(k == KIN - 1),
                                    )
                                st = xstpool.tile([128, 512], f16, tag="xst")
                                if m % 2 == 0:
                                    nc.scalar.activation(
                                        st, ps, Ident, bias=bx[l, d][:, m : m + 1]
                                    )
                                else:
                                    nc.vector.tensor_scalar_add(
                                        st, ps, bx[l, d][:, m : m + 1]
                                    )
                                nc.sync.dma_start(
                                    out=xg[l, d][p, m * 128 : (m + 1) * 128, :], in_=st
                                )

                # -------- phase B: recurrence for layer l --------
                with (
                    tc.tile_pool(name=f"whh{l}", bufs=1) as whpool,
                    tc.tile_pool(name=f"pg{l}", bufs=2, space="PSUM") as pgpool,
                    tc.tile_pool(name=f"pt{l}", bufs=2, space="PSUM") as ptpool,
                ):
                    whh = {}
                    for d in range(D):
                        w = whpool.tile([128, NCH * G3], f16, tag=f"whh{d}")
                        nc.sync.dma_start(
                            out=w.rearrange("p (k n) -> p k n", k=NCH),
                            in_=whh_in[l, d].rearrange("(k p) n -> p k n", p=128),
                        )
                        whh[d] = w

                    # per-dir rolling state refs, always 3D APs [p, c, b]
                    h32_prev = {}
                    h16_prev = {}
                    for d in range(D):
                        a = hpool.tile([128, NCH * BL], f32, tag=f"h32i{d}")
                        nc.vector.memset(a, 0.0)
                        b = hpool.tile([128, NCH * BL], f16, tag=f"h16i{d}")
                        nc.vector.memset(b, 0.0)
                        h32_prev[d] = a.rearrange("p (c b) -> p c b", b=BL)
                        h16_prev[d] = b.rearrange("p (c b) -> p c b", b=BL)

                    ydt = f16 if l == 0 else f32
                    ystage = {d: None for d in range(D)}
                    xw = {d: None for d in range(D)}

                    for t in range(S):
                        tt = t % PIECE
                        w = t // PIECE
                        if tt == 0:
                            for d in range(D):
                                # flush previous window's y-stage
                                if w > 0:
                                    self_w = w - 1
                                    dst_w = self_w if d == 0 else NP - 1 - self_w
                                    dst = (y0T[d] if l == 0 else y1T_out[d]).rearrange(
                                        "(c p) n -> p c n", p=128
                                    )[:, :, dst_w * PB : (dst_w + 1) * PB]
                                    nc.sync.dma_start(
                                        out=dst,
                                        in_=ystage[d].rearrange("p (c n) -> p c n", c=NCH),
                                    )
                                yst_t = ypool.tile([128, NCH * PB], ydt, tag=f"yst{d}")
                                ystage[d] = yst_t
                                pc = w if d == 0 else NP - 1 - w
                                nxw = xwpool.tile([128, NJ * PB], f16, tag=f"xw{d}")
                                nc.sync.dma_start(
                                    out=nxw.rearrange("p (j n) -> p j n", j=NJ),
                                    in_=xg[l, d][pc].rearrange("(j p) n -> p j n", p=128),
                                )
                                xw[d] = nxw

                        for d in range(D):
                            slot = tt if d == 0 else PIECE - 1 - tt
                            # ---- strip matmuls: gh = h^T @ whh (quarter-permuted)
                            gh = pgpool.tile([128, 384], f32, tag=f"gh{d}")
                            wh3 = whh[d].rearrange("p (k n) -> p k n", k=NCH)
                            for k in range(NCH):
                                for c in range(4):
                                    nc.tensor.matmul(
                                        gh[32 * c : 32 * c + BL, :],
                                        h16_prev[d][:, k, :],
                                        wh3[:, k, c * 384 : (c + 1) * 384],
                                        start=(k == 0),
                                        stop=(k == NCH - 1),
                                        tile_position=(0, 32 * c),
                                    )
                            ghs = spool.tile([128, 384], f16, tag=f"ghs{d}")
                            nc.scalar.copy(out=ghs, in_=gh)
                            ghT = ptpool.tile([128, 384], f16, tag=f"ghT{d}")
                            for g in range(3):
                                nc.tensor.transpose(
                                    ghT[:, g * 128 : (g + 1) * 128],
                                    ghs[:, g * 128 : (g + 1) * 128],
                                    ident,
                                )
                            # ---- elementwise; slots (32c + b), b < BL valid
                            ghT3 = ghT.rearrange("p (j s) -> p j s", s=32)
                            xw3 = xw[d].rearrange("p (j n) -> p j n", j=NJ)
                            rz = spool.tile([128, 8 * BL], f16, tag=f"rz{d}")
                            nc.vector.tensor_add(
                                rz.rearrange("p (j b) -> p j b", b=BL),
                                ghT3[:, 0:8, 0:BL],
                                xw3[:, 0:8, slot * BL : (slot + 1) * BL],
                            )
                            rzs = spool.tile([128, 8 * BL], f16, tag=f"rzs{d}")
                            nc.scalar.activation(rzs, rz, Sig)
                            rzs3 = rzs.rearrange("p (j b) -> p j b", b=BL)
                            rhn = spool.tile([128, NCH * BL], f16, tag=f"rhn{d}")
                            nc.vector.scalar_tensor_tensor(
                                out=rhn.rearrange("p (j b) -> p j b", b=BL),
                                in0=ghT3[:, 8:12, 0:BL],
                                scalar=0.0,
                                in1=rzs3[:, 0:4, :],
                                op0=mybir.AluOpType.add,
                                op1=mybir.AluOpType.mult,
                            )
                            npre = spool.tile([128, NCH * BL], f16, tag=f"npre{d}")
                            nc.vector.tensor_add(
                                npre.rearrange("p (j b) -> p j b", b=BL),
                                rhn.rearrange("p (j b) -> p j b", b=BL),
                                xw3[:, 8:12, slot * BL : (slot + 1) * BL],
                            )
                            nt = spool.tile([128, NCH * BL], f16, tag=f"nt{d}")
                            nc.scalar.activation(nt, npre, Tanh)
                            hmn = spool.tile([128, NCH * BL], f32, tag=f"hmn{d}")
                            nc.vector.tensor_sub(
                                hmn.rearrange("p (j b) -> p j b", b=BL),
                                h32_prev[d],
                                nt.rearrange("p (j b) -> p j b", b=BL),
                            )
                            zhm = spool.tile([128, NCH * BL], f32, tag=f"zhm{d}")
                            nc.vector.tensor_mul(
                                zhm.rearrange("p (j b) -> p j b", b=BL),
                                rzs3[:, 4:8, :],
                                hmn.rearrange("p (j b) -> p j b", b=BL),
                            )
                            yslot = ystage[d].rearrange(
                                "p (c n) -> p c n", c=NCH
                            )[:, :, slot * BL : (slot + 1) * BL]
                            if l == 0:
                                h32 = hpool.tile([128, NCH * BL], f32, tag=f"h32_{d}")
                                h32v = h32.rearrange("p (c b) -> p c b", b=BL)
                                nc.vector.tensor_add(
                                    h32v,
                                    nt.rearrange("p (c b) -> p c b", b=BL),
                                    zhm.rearrange("p (c b) -> p c b", b=BL),
                                )
                                nc.vector.tensor_copy(yslot, h32v)
                                # mm stationary reads the y-stage fp16 slot next step
                                h16_prev[d] = yslot
                                h32_prev[d] = h32v
                            else:
                                nc.vector.tensor_add(
                                    yslot,
                                    nt.rearrange("p (c b) -> p c b", b=BL),
                                    zhm.rearrange("p (c b) -> p c b", b=BL),
                                )
                                h16 = hpool.tile([128, NCH * BL], f16, tag=f"h16_{d}")
                                h16v = h16.rearrange("p (c b) -> p c b", b=BL)
                                nc.vector.tensor_copy(h16v, yslot)
                                h16_prev[d] = h16v
                                h32_prev[d] = yslot

                            if t == S - 1:
                                nc.sync.dma_start(
                                    out=hN_out[l * D + d].rearrange("(c p) b -> p c b", p=128),
                                    in_=h32_prev[d],
                                )
                    # flush final window
                    for d in range(D):
                        dst_w = NP - 1 if d == 0 else 0
                        dst = (y0T[d] if l == 0 else y1T_out[d]).rearrange(
                            "(c p) n -> p c n", p=128
                        )[:, :, dst_w * PB : (dst_w + 1) * PB]
                        nc.sync.dma_start(
                            out=dst, in_=ystage[d].rearrange("p (c n) -> p c n", c=NCH)
                        )

    nc.compile()
    return nc


def _host_prepare(x, params):
    """Build per-core input maps from full inputs."""
    x = np.asarray(x, dtype=np.float32)
    common = {}
    for l in range(L):
        for d in range(D):
            w_ih, w_hh, b_ih, b_hh = [np.asarray(a, dtype=np.float32) for a in params[l][d]]
            w_hhT = w_hh.T  # [H, 3H] cols gate-major
            quarters = []
            for c in range(NCH):
                for g in range(3):
                    quarters.append(w_hhT[:, g * H + c * 128 : g * H + (c + 1) * 128])
            common[f"whh_{l}_{d}"] = np.concatenate(quarters, axis=1).astype(np.float16)
            common[f"wih_{l}_{d}"] = w_ih.T.astype(np.float16)
            bxv = np.zeros((128, NJ), np.float32)
            for g in range(3):
                for c in range(NCH):
                    j = g * 4 + c
                    rows = slice(g * H + c * 128, g * H + (c + 1) * 128)
                    bxv[:, j] = b_ih[rows] + (b_hh[rows] if g < 2 else 0.0)
            common[f"bx_{l}_{d}"] = bxv
            assert np.all(b_hh[2 * H :] == 0.0), "nonzero b_hh[n] not supported"
    common["ident"] = np.eye(128, dtype=np.float16)

    in_maps = []
    for g in range(NCORES):
        m = dict(common)
        xs = x[g * BL : (g + 1) * BL]  # [BL, S, I]
        m["xT"] = np.ascontiguousarray(xs.transpose(2, 1, 0).reshape(I_IN, S * BL)).astype(
            np.float16
        )
        in_maps.append(m)
    return in_maps


def kernel(x, params):
    from concourse.bass_utils import run_bass_kernel_spmd

    if "nc" not in _CACHE:
        _CACHE["nc"] = _build()
    nc = _CACHE["nc"]
    in_maps = _host_prepare(x, params)
    res = run_bass_kernel_spmd(nc, in_maps, core_ids=list(range(NCORES)))
    _CACHE["last_results"] = res

    y = np.zeros((B_TOT, S, D * H), np.float32)
    hidden = np.zeros((L * D, B_TOT, H), np.float32)
    for g in range(NCORES):
        r = res.results[g]
        y1T = r["y1T"]  # [D, H, S*BL]
        for d in range(D):
            # y[b, t, d*H + h] = y1T[d, h, t*BL + b]
            y[g * BL : (g + 1) * BL, :, d * H : (d + 1) * H] = (
                y1T[d].reshape(H, S, BL).transpose(2, 1, 0)
            )
        hN = r["hN"]  # [L*D, H, BL]
        hidden[:, g * BL : (g + 1) * BL, :] = hN.transpose(0, 2, 1)
    return y, hidden
